# revision 1
# baseline (speedup 1.0000x reference)
"""Trainium2 Bass kernel for nn_KanBoard768 (KAN network forward pass).

Data-parallel across 8 NeuronCores: batch 32768 -> 4096 rows/core, weights
replicated, no collectives.

Math: the cubic B-spline bases are reformulated as truncated powers,
    N(u - j) = (1/6) * sum_r (-1)^r C(4,r) relu(u - j - r)^3
so the spline matmul becomes  sum_{e,s} D[o,e,s] * relu(u_e - s)^3  with the
binomial transform folded into D on the host.  The relu-cube features are
produced by a fused custom DVE op (mul, sub, relu, sq, mul = 5 ALU stages)
reading the hidden activations straight from PSUM, with the grid transform
u = (x + ft_b - g0)/h folded into the op's scale and per-partition shift.
"""

import numpy as np

# --- problem constants (hardcoded; kernel.py must be self-contained) ---
GRID_SIZE, SPLINE_ORDER = 5, 3
H = 2.0 / GRID_SIZE                    # 0.4
G0 = -SPLINE_ORDER * H - 1.0           # -2.2
INV_H = 1.0 / H                        # 2.5 (exact in fp32)
NB = GRID_SIZE + SPLINE_ORDER          # 8 bases per edge
NS = GRID_SIZE + 2 * SPLINE_ORDER + 1  # 12 truncated-power shifts
B, IN_FT, HID = 32768, 768, 128
NCORES = 8
BC = B // NCORES                       # 4096 rows per core
NT = 512                               # batch tile (one PSUM bank of fp32)
NBT = BC // NT                         # 8 batch tiles per core
KT_FT = IN_FT // 128                   # 6 contraction tiles for the ft layer

_CACHE = {}


def _register_relu_cube():
    import concourse.dve_ops as dve_ops
    from concourse.dve_spec import Spec, Src0, C0, C2, relu, sq, lower
    from concourse.dve_uop import DveOpSpec

    name = "RELU_CUBE_AFF_ANT"
    for op in dve_ops.OPS:
        if op.name == name:
            return op
    r = relu(Src0 * C2 - C0)
    spec = Spec(
        body=sq(r) * r,
        reference=lambda in0, in1, s0, s1, imm2: np.maximum(
            in0.astype(np.float32) * imm2 - s0, 0.0
        )
        ** 3,
    )
    row = dve_ops._CUSTOM_DVE_ROW_BASE + len(dve_ops.OPS)
    assert row < 0x20
    shas = {}
    for ver in ("v3", "v4"):
        try:
            shas[ver] = DveOpSpec(
                name=name, opcode=row, uops=lower(spec, ver=ver), rd1_en=False
            ).sha(ver)
        except Exception:
            pass
    op = dve_ops.DveOp(name, spec, subdim=False, uops_sha=shas)
    dve_ops.OPS.append(op)
    dve_ops._SUB_OPCODE_FOR_NAME[name] = row
    dve_ops.CUSTOM_DVE_SPECS[name] = spec
    return op


def _build_module():
    if "nc" in _CACHE:
        return _CACHE["nc"]
    from contextlib import ExitStack

    import concourse.bass as bass
    import concourse.mybir as mybir
    import concourse.tile as tile
    from concourse import bacc

    RELU_CUBE = _register_relu_cube()
    AF = mybir.ActivationFunctionType
    f32 = mybir.dt.float32

    nc = bacc.Bacc("TRN2", target_bir_lowering=False, debug=False)

    stmT = nc.dram_tensor("stm_t", (IN_FT, BC), f32, kind="ExternalInput").ap()
    nstmT = nc.dram_tensor("nstm_t", (IN_FT, BC), f32, kind="ExternalInput").ap()
    wft = nc.dram_tensor("wft", (KT_FT, 128, 128), f32, kind="ExternalInput").ap()
    d1 = nc.dram_tensor("d1", (2 * NS, 128, 128), f32, kind="ExternalInput").ap()
    b1 = nc.dram_tensor("b1", (2, 128, 128), f32, kind="ExternalInput").ap()
    d2 = nc.dram_tensor("d2", (NS + 1, 128, 1), f32, kind="ExternalInput").ap()
    sh1 = nc.dram_tensor("sh1", (128, NS), f32, kind="ExternalInput").ap()
    ftb = nc.dram_tensor("ftb", (128, 1), f32, kind="ExternalInput").ap()
    out_d = nc.dram_tensor("out", (1, BC), f32, kind="ExternalOutput").ap()

    with tile.TileContext(nc) as tc, ExitStack() as ctx:
        wpool = ctx.enter_context(tc.tile_pool(name="weights", bufs=1))
        inpool = ctx.enter_context(tc.tile_pool(name="inp", bufs=3))
        spool = ctx.enter_context(tc.tile_pool(name="silu", bufs=3))
        fpool = ctx.enter_context(tc.tile_pool(name="feats", bufs=32))
        opool = ctx.enter_context(tc.tile_pool(name="outb", bufs=1))
        pspool = ctx.enter_context(tc.tile_pool(name="ps", bufs=2, space="PSUM"))
        popool = ctx.enter_context(tc.tile_pool(name="pso", bufs=2, space="PSUM"))

        wft_sb = wpool.tile([128, KT_FT, 128], f32)
        nc.sync.dma_start(wft_sb[:], wft.rearrange("k p m -> p k m"))
        d1_sb = wpool.tile([128, 2 * NS, 128], f32)
        nc.sync.dma_start(d1_sb[:], d1.rearrange("k p m -> p k m"))
        b1_sb = wpool.tile([128, 2, 128], f32)
        nc.sync.dma_start(b1_sb[:], b1.rearrange("k p m -> p k m"))
        d2_sb = wpool.tile([128, NS + 1, 1], f32)
        nc.sync.dma_start(d2_sb[:], d2.rearrange("k p m -> p k m"))
        sh1_sb = wpool.tile([128, NS], f32)
        nc.sync.dma_start(sh1_sb[:], sh1[:])
        ftb_sb = wpool.tile([128, 1], f32)
        nc.sync.dma_start(ftb_sb[:], ftb[:])

        outbuf = opool.tile([1, BC], f32)
        out_sig = opool.tile([1, BC], f32)

        stmT_r = stmT.rearrange("(k p) n -> p k n", p=128)
        nstmT_r = nstmT.rearrange("(k p) n -> p k n", p=128)

        for bt in range(NBT):
            sl = bass.ts(bt, NT)
            xs = inpool.tile([128, KT_FT, NT], f32, tag="xs")
            nc.sync.dma_start(xs[:], stmT_r[:, :, sl])
            xn = inpool.tile([128, KT_FT, NT], f32, tag="xn")
            nc.sync.dma_start(xn[:], nstmT_r[:, :, sl])

            ps_s = pspool.tile([128, NT], f32, tag="ps_s")
            ps_n = pspool.tile([128, NT], f32, tag="ps_n")
            for k in range(KT_FT):
                nc.tensor.matmul(
                    ps_s[:], wft_sb[:, k, :], xs[:, k, :],
                    start=(k == 0), stop=(k == KT_FT - 1),
                )
            for k in range(KT_FT):
                nc.tensor.matmul(
                    ps_n[:], wft_sb[:, k, :], xn[:, k, :],
                    start=(k == 0), stop=(k == KT_FT - 1),
                )

            silu_s = spool.tile([128, NT], f32, tag="sl_s")
            nc.scalar.activation(silu_s[:], ps_s[:], AF.Silu, bias=ftb_sb[:])
            silu_n = spool.tile([128, NT], f32, tag="sl_n")
            nc.scalar.activation(silu_n[:], ps_n[:], AF.Silu, bias=ftb_sb[:])

            ps_h2 = pspool.tile([128, NT], f32, tag="ps_h2")
            mmi = 0
            for half, ps_x in ((0, ps_s), (1, ps_n)):
                for s in range(NS):
                    f = fpool.tile([128, NT], f32, tag="feat")
                    nc.vector._custom_dve(
                        RELU_CUBE, out=f[:], in0=ps_x[:],
                        s0=sh1_sb[:, s : s + 1], imm2=INV_H,
                    )
                    nc.tensor.matmul(
                        ps_h2[:], d1_sb[:, half * NS + s, :], f[:],
                        start=(mmi == 0), stop=False,
                    )
                    mmi += 1
            nc.tensor.matmul(ps_h2[:], b1_sb[:, 0, :], silu_s[:], start=False, stop=False)
            nc.tensor.matmul(ps_h2[:], b1_sb[:, 1, :], silu_n[:], start=False, stop=True)

            silu2 = spool.tile([128, NT], f32, tag="sl2")
            nc.scalar.activation(silu2[:], ps_h2[:], AF.Silu, bias=0.0)

            ps_o = popool.tile([1, NT], f32, tag="ps_o")
            for s in range(NS):
                f2 = fpool.tile([128, NT], f32, tag="feat")
                nc.vector._custom_dve(
                    RELU_CUBE, out=f2[:], in0=ps_h2[:],
                    s0=float(s + G0 * INV_H), imm2=INV_H,
                )
                nc.tensor.matmul(
                    ps_o[:], d2_sb[:, s, :], f2[:], start=(s == 0), stop=False
                )
            nc.tensor.matmul(ps_o[:], d2_sb[:, NS, :], silu2[:], start=False, stop=True)

            nc.vector.tensor_copy(outbuf[:, sl], ps_o[:])

        nc.scalar.activation(out_sig[:], outbuf[:], AF.Sigmoid, bias=0.0)
        nc.sync.dma_start(out_d[:], out_sig[:])

    nc.compile()
    _CACHE["nc"] = nc
    return nc


def _make_D(spline_w):
    # spline_w: (out, in, NB) -> D: (out, in, NS) via the binomial transform
    out, inn, nb = spline_w.shape
    C4 = np.array([1.0, -4.0, 6.0, -4.0, 1.0], dtype=np.float64) / 6.0
    D = np.zeros((out, inn, NS), dtype=np.float64)
    sw = spline_w.astype(np.float64)
    for j in range(NB):
        for r in range(5):
            D[:, :, j + r] += C4[r] * sw[:, :, j]
    return D.astype(np.float32)


def _host_prep(inputs):
    stm = np.asarray(inputs["stm"], dtype=np.float32)
    nstm = np.asarray(inputs["nstm"], dtype=np.float32)
    ft_w = np.asarray(inputs["ft_w"], dtype=np.float32)
    ft_b = np.asarray(inputs["ft_b"], dtype=np.float32)
    w1b = np.asarray(inputs["kan1_base_w"], dtype=np.float32)
    w1s = np.asarray(inputs["kan1_spline_w"], dtype=np.float32)
    w2b = np.asarray(inputs["kan2_base_w"], dtype=np.float32)
    w2s = np.asarray(inputs["kan2_spline_w"], dtype=np.float32)

    stmT = np.ascontiguousarray(stm.T)    # (768, B)
    nstmT = np.ascontiguousarray(nstm.T)

    # ft layer: lhsT[k, m] = ft_w[m, k] -> tiles (KT, 128, 128)
    wft_np = np.ascontiguousarray(ft_w.T.reshape(KT_FT, 128, HID))

    # kan1 spline: D1 (128, 256, NS); lhsT tile [e, o] per (half, s)
    D1 = _make_D(w1s)
    d1_np = np.empty((2 * NS, 128, 128), dtype=np.float32)
    for half in range(2):
        for s in range(NS):
            d1_np[half * NS + s] = D1[:, half * 128 : (half + 1) * 128, s].T
    b1_np = np.stack([w1b[:, :128].T, w1b[:, 128:].T]).astype(np.float32)

    # kan2: D2 (1, 128, NS) -> columns [e2, 1]; last slot = base weights
    D2 = _make_D(w2s)
    d2_np = np.empty((NS + 1, 128, 1), dtype=np.float32)
    for s in range(NS):
        d2_np[s, :, 0] = D2[0, :, s]
    d2_np[NS, :, 0] = w2b[0, :]

    # per-partition shift vector for layer-1 features: u = x*INV_H + bv,
    # t = u - s = x*INV_H - (s - bv);  bv = (ft_b - G0)/H
    bv = (ft_b.astype(np.float64) - G0) / H
    sh1_np = (
        np.arange(NS, dtype=np.float64)[None, :] - bv[:, None]
    ).astype(np.float32)
    ftb_np = ft_b.reshape(128, 1).astype(np.float32)

    weights = dict(
        wft=wft_np, d1=d1_np, b1=b1_np, d2=d2_np, sh1=sh1_np, ftb=ftb_np
    )
    return stmT, nstmT, weights


def kernel(**inputs):
    from concourse.bass_utils import run_bass_kernel_spmd

    nc = _build_module()
    stmT, nstmT, weights = _host_prep(inputs)

    in_maps = []
    for c in range(NCORES):
        sl = slice(c * BC, (c + 1) * BC)
        m = {
            "stm_t": np.ascontiguousarray(stmT[:, sl]),
            "nstm_t": np.ascontiguousarray(nstmT[:, sl]),
        }
        m.update(weights)
        in_maps.append(m)

    res = run_bass_kernel_spmd(nc, in_maps, core_ids=list(range(NCORES)))
    out = np.concatenate([r["out"].reshape(-1) for r in res.results])
    return out.reshape(B, 1).astype(np.float32)


if __name__ == "__main__":
    rng = np.random.default_rng(0)
    nb = NB
    fake = {
        "stm": rng.random((B, IN_FT), dtype=np.float32),
        "nstm": rng.random((B, IN_FT), dtype=np.float32),
        "ft_w": (rng.standard_normal((HID, IN_FT)) * 0.02).astype(np.float32),
        "ft_b": np.zeros(HID, np.float32),
        "kan1_base_w": (rng.standard_normal((HID, 2 * HID)) * 0.05).astype(np.float32),
        "kan1_spline_w": (rng.standard_normal((HID, 2 * HID, nb)) * 0.05).astype(np.float32),
        "kan2_base_w": (rng.standard_normal((1, HID)) * 0.05).astype(np.float32),
        "kan2_spline_w": (rng.standard_normal((1, HID, nb)) * 0.05).astype(np.float32),
    }
    out = kernel(**fake)
    print("kernel out", out.shape, out.dtype, out[:5, 0])



# revision 2
# speedup vs baseline: 1.0140x; 1.0140x over previous
"""Trainium2 Bass kernel for nn_KanBoard768 (KAN forward).

Data-parallel across 8 NeuronCores: batch 32768 -> 4096 rows/core, weights
replicated, no collectives.

Math: cubic B-spline basis evaluated exactly via the bounded cardinal form
  6*B(t) = m^3 - 4*relu(m-1)^3,   m = relu(min(t, 4-t)),  t = u - j,
  u = INV_H*x + bv.
m in [0,2], 6B in [0,4]: features and spline matmuls are fp16-safe.

Engine split (v3):
 - PE: all matmuls fp16 (1 cyc/row).
 - DVE: TENT_PSUM_PAGED computes m for shifts j0..7 of a layer in ONE
   paged instruction (PageIdx), reading PSUM fp32 directly with the
   grid transform folded into per-partition constants; COMB turns the
   whole m tile into 6B in one flat pass.
 - ACT: shifts 0..j0-1 via Abs(from PSUM) + Relu(2-a) pairs, plus silu,
   all from the single `silu_and_others` act table set (no reloads).
   Sigmoid deferred to ONE final pass (one table switch per kernel).
"""

import numpy as np

GRID_SIZE, SPLINE_ORDER = 5, 3
H = 2.0 / GRID_SIZE                    # 0.4
G0 = -SPLINE_ORDER * H - 1.0           # -2.2
INV_H = 1.0 / H                        # 2.5
NB = GRID_SIZE + SPLINE_ORDER          # 8 basis functions per edge
B, IN_FT, HID = 32768, 768, 128
NCORES = 8
BC = B // NCORES                       # 4096 rows per core
NT = 512
NBT = BC // NT                         # 8 batch tiles per core
KT_FT = IN_FT // 128
CBRT4 = float(4.0 ** (1.0 / 3.0))
BV2 = -G0 * INV_H                      # 5.5 (layer-2 u bias)

# shifts j < J0 go to the ACT engine (Abs+Relu), j >= J0 to the DVE
# (single paged TENT instruction). Tuned for DVE/ACT balance.
J0_L1 = 5
J0_L2 = 4

_CACHE = {}


def _register_ops():
    import concourse.dve_ops as dve_ops
    from concourse.dve_spec import (
        Spec, Src0, C0, C1, C2, Zero, One, PageIdx, relu, sq, minn, lower,
    )
    from concourse.dve_uop import DveOpSpec

    _pg = PageIdx(Zero, One)

    def tent_ref(in0, in1, s0, s1, imm2):
        x = in0.astype(np.float32)
        S = x.shape[1] if x.ndim == 3 else 1
        pg = np.arange(S, dtype=np.float32).reshape(1, S, 1)
        s0 = np.asarray(s0, np.float32).reshape(-1, 1, 1)
        s1 = np.asarray(s1, np.float32).reshape(-1, 1, 1)
        r = x * imm2 - pg
        return np.maximum(np.minimum(r - s0, s1 - r), 0.0)

    defs = {
        # m_j = relu(min(r - C0, C1 - r)), r = x*C2 - pg, pg = page idx
        "KTENTP_ANT": (
            Spec(
                body=(lambda r: relu(minn(r - C0, C1 - r)))(Src0 * C2 - _pg),
                reference=tent_ref,
            ),
            True,
        ),
        # 6B = m^3 - relu(c*m - c)^3  (c = cbrt4)
        "KCOMB_ANT": (
            Spec(
                body=(lambda m3, w: m3 - sq(w) * w)(
                    (lambda m2: m2 * Src0)(sq(Src0)),
                    relu(Src0 * C2 - C0),
                ),
                reference=lambda in0, in1, s0, s1, imm2: (
                    in0.astype(np.float32) ** 3
                    - np.maximum(in0.astype(np.float32) * imm2 - s0, 0.0) ** 3
                ),
            ),
            False,
        ),
    }
    ops = {}
    for name, (spec, subdim) in defs.items():
        found = None
        for op in dve_ops.OPS:
            if op.name == name:
                found = op
        if found is None:
            row = dve_ops._CUSTOM_DVE_ROW_BASE + len(dve_ops.OPS)
            assert row < 0x20
            shas = {}
            for ver in ("v3", "v4"):
                try:
                    shas[ver] = DveOpSpec(
                        name=name, opcode=row, uops=lower(spec, ver=ver), rd1_en=False
                    ).sha(ver)
                except Exception:
                    pass
            found = dve_ops.DveOp(name, spec, subdim=subdim, uops_sha=shas)
            dve_ops.OPS.append(found)
            dve_ops._SUB_OPCODE_FOR_NAME[name] = row
            dve_ops.CUSTOM_DVE_SPECS[name] = spec
        ops[name] = found
    return ops


def _build_module():
    if "nc" in _CACHE:
        return _CACHE["nc"]
    from contextlib import ExitStack

    import concourse.bass as bass
    import concourse.mybir as mybir
    import concourse.tile as tile
    from concourse import bacc

    ops = _register_ops()
    TENTP, COMB = ops["KTENTP_ANT"], ops["KCOMB_ANT"]
    AF = mybir.ActivationFunctionType
    f32, f16 = mybir.dt.float32, mybir.dt.float16

    nc = bacc.Bacc("TRN2", target_bir_lowering=False, debug=False)

    stmT = nc.dram_tensor("stm_t", (IN_FT, BC), f16, kind="ExternalInput").ap()
    nstmT = nc.dram_tensor("nstm_t", (IN_FT, BC), f16, kind="ExternalInput").ap()
    wft = nc.dram_tensor("wft", (KT_FT, 128, 128), f16, kind="ExternalInput").ap()
    d1 = nc.dram_tensor("d1", (2 * NB, 128, 128), f16, kind="ExternalInput").ap()
    b1 = nc.dram_tensor("b1", (2, 128, 128), f16, kind="ExternalInput").ap()
    d2 = nc.dram_tensor("d2", (NB + 1, 128, 1), f16, kind="ExternalInput").ap()
    # constant tables: [128, 8] per use
    c0l1 = nc.dram_tensor("c0l1", (128, NB), f32, kind="ExternalInput").ap()
    c1l1 = nc.dram_tensor("c1l1", (128, NB), f32, kind="ExternalInput").ap()
    abl1 = nc.dram_tensor("abl1", (128, NB), f32, kind="ExternalInput").ap()
    c0l2 = nc.dram_tensor("c0l2", (128, NB), f32, kind="ExternalInput").ap()
    c1l2 = nc.dram_tensor("c1l2", (128, NB), f32, kind="ExternalInput").ap()
    abl2 = nc.dram_tensor("abl2", (128, NB), f32, kind="ExternalInput").ap()
    two = nc.dram_tensor("two", (128, 1), f32, kind="ExternalInput").ap()
    ftb = nc.dram_tensor("ftb", (128, 1), f32, kind="ExternalInput").ap()
    out_d = nc.dram_tensor("out", (1, BC), f16, kind="ExternalOutput").ap()

    with tile.TileContext(nc) as tc, ExitStack() as ctx:
        wpool = ctx.enter_context(tc.tile_pool(name="weights", bufs=1))
        inpool = ctx.enter_context(tc.tile_pool(name="inp", bufs=2))
        apool = ctx.enter_context(tc.tile_pool(name="abs", bufs=1))
        slpool = ctx.enter_context(tc.tile_pool(name="silu", bufs=2))
        mpool = ctx.enter_context(tc.tile_pool(name="mt", bufs=2))
        fpool = ctx.enter_context(tc.tile_pool(name="feat", bufs=2))
        opool = ctx.enter_context(tc.tile_pool(name="outb", bufs=1))
        pspool = ctx.enter_context(tc.tile_pool(name="ps", bufs=2, space="PSUM"))
        popool = ctx.enter_context(tc.tile_pool(name="pso", bufs=2, space="PSUM"))

        # prefetch the first input chunks before the bulky weight DMAs
        stmT_r = stmT.rearrange("(k p) n -> p k n", p=128)
        nstmT_r = nstmT.rearrange("(k p) n -> p k n", p=128)
        pre_xs = inpool.tile([128, KT_FT, NT // 2], f16, tag="xsh", bufs=1)
        nc.sync.dma_start(pre_xs[:], stmT_r[:, :, 0 : NT // 2])
        pre_xn = inpool.tile([128, KT_FT, NT // 2], f16, tag="xnh", bufs=1)
        nc.sync.dma_start(pre_xn[:], nstmT_r[:, :, 0 : NT // 2])

        wft_sb = wpool.tile([128, KT_FT, 128], f16)
        nc.sync.dma_start(wft_sb[:], wft.rearrange("k p m -> p k m"))
        d1_sb = wpool.tile([128, 2 * NB, 128], f16)
        nc.sync.dma_start(d1_sb[:], d1.rearrange("k p m -> p k m"))
        b1_sb = wpool.tile([128, 2, 128], f16)
        nc.sync.dma_start(b1_sb[:], b1.rearrange("k p m -> p k m"))
        d2_sb = wpool.tile([128, NB + 1, 1], f16)
        nc.sync.dma_start(d2_sb[:], d2.rearrange("k p m -> p k m"))
        c0l1_sb = wpool.tile([128, NB], f32)
        nc.sync.dma_start(c0l1_sb[:], c0l1[:])
        c1l1_sb = wpool.tile([128, NB], f32)
        nc.sync.dma_start(c1l1_sb[:], c1l1[:])
        abl1_sb = wpool.tile([128, NB], f32)
        nc.sync.dma_start(abl1_sb[:], abl1[:])
        c0l2_sb = wpool.tile([128, NB], f32)
        nc.sync.dma_start(c0l2_sb[:], c0l2[:])
        c1l2_sb = wpool.tile([128, NB], f32)
        nc.sync.dma_start(c1l2_sb[:], c1l2[:])
        abl2_sb = wpool.tile([128, NB], f32)
        nc.sync.dma_start(abl2_sb[:], abl2[:])
        two_sb = wpool.tile([128, 1], f32)
        nc.sync.dma_start(two_sb[:], two[:])
        ftb_sb = wpool.tile([128, 1], f32)
        nc.sync.dma_start(ftb_sb[:], ftb[:])

        outbuf = opool.tile([1, BC], f16)
        outsig = opool.tile([1, BC], f16)

        # chunk schedule: first and last full tiles split in half to
        # shorten pipeline fill and drain.
        chunks = [(0, NT // 2), (NT // 2, NT // 2)]
        chunks += [(i * NT, NT) for i in range(1, NBT - 1)]
        chunks += [((NBT - 1) * NT, NT // 2), ((NBT - 1) * NT + NT // 2, NT // 2)]
        for (c0, W) in chunks:
            tw = "h" if W != NT else ""
            sl = slice(c0, c0 + W)
            if c0 == 0:
                xs, xn = pre_xs, pre_xn
            else:
                xs = inpool.tile([128, KT_FT, W], f16, tag="xs" + tw, bufs=(1 if tw else None))
                nc.sync.dma_start(xs[:], stmT_r[:, :, sl])
                xn = inpool.tile([128, KT_FT, W], f16, tag="xn" + tw, bufs=(1 if tw else None))
                nc.sync.dma_start(xn[:], nstmT_r[:, :, sl])

            ps_s_t = pspool.tile([128, NT], f32, tag="ps_s")
            ps_n_t = pspool.tile([128, NT], f32, tag="ps_n")
            ps_s = ps_s_t[:, 0:W]
            ps_n = ps_n_t[:, 0:W]
            for k in range(KT_FT):
                nc.tensor.matmul(
                    ps_s, wft_sb[:, k, :], xs[:, k, :],
                    start=(k == 0), stop=(k == KT_FT - 1),
                )
            for k in range(KT_FT):
                nc.tensor.matmul(
                    ps_n, wft_sb[:, k, :], xn[:, k, :],
                    start=(k == 0), stop=(k == KT_FT - 1),
                )

            # layer-1 m/6B tiles, split into a DVE part (shifts J0..7, one
            # paged TENT) and an ACT part (shifts 0..J0-1, Abs+merged Relu)
            # so the DVE-side COMB never waits on the ACT engine.
            m1d = mpool.tile([128, NB - J0_L1, 2 * W], f16, tag="m1d" + tw, bufs=(1 if tw else None))
            m1a = mpool.tile([128, J0_L1, 2 * W], f16, tag="m1a" + tw, bufs=(1 if tw else None))
            a1 = apool.tile([128, J0_L1, 2 * W], f16, tag="a1" + tw, bufs=(1 if tw else None))
            for j in range(J0_L1):
                nc.scalar.activation(
                    a1[:, j, 0:W], ps_s, AF.Abs,
                    bias=abl1_sb[:, j : j + 1], scale=INV_H,
                )
                nc.scalar.activation(
                    a1[:, j, W : 2 * W], ps_n, AF.Abs,
                    bias=abl1_sb[:, j : j + 1], scale=INV_H,
                )
            nc.scalar.activation(
                m1a[:], a1[:], AF.Relu, bias=two_sb[:], scale=-1.0
            )
            # DVE path: shifts J0_L1..7 in one paged instruction per half
            npg = NB - J0_L1
            in_s = ps_s.unsqueeze(1).broadcast_to((128, npg, W))
            in_n = ps_n.unsqueeze(1).broadcast_to((128, npg, W))
            nc.vector._custom_dve(
                TENTP, out=m1d[:, :, 0:W], in0=in_s,
                s0=c0l1_sb[:, J0_L1 : J0_L1 + 1], s1=c1l1_sb[:, J0_L1 : J0_L1 + 1],
                imm2=INV_H,
            )
            nc.vector._custom_dve(
                TENTP, out=m1d[:, :, W : 2 * W], in0=in_n,
                s0=c0l1_sb[:, J0_L1 : J0_L1 + 1], s1=c1l1_sb[:, J0_L1 : J0_L1 + 1],
                imm2=INV_H,
            )
            f1d = fpool.tile([128, NB - J0_L1, 2 * W], f16, tag="f1d" + tw, bufs=(1 if tw else None))
            nc.vector._custom_dve(
                COMB, out=f1d[:], in0=m1d[:], s0=CBRT4, imm2=CBRT4,
            )
            f1a = fpool.tile([128, J0_L1, 2 * W], f16, tag="f1a" + tw, bufs=(1 if tw else None))
            nc.vector._custom_dve(
                COMB, out=f1a[:], in0=m1a[:], s0=CBRT4, imm2=CBRT4,
            )
            silu_s = slpool.tile([128, W], f16, tag="sl_s" + tw, bufs=(1 if tw else None))
            nc.scalar.activation(silu_s[:], ps_s, AF.Silu, bias=ftb_sb[:])
            silu_n = slpool.tile([128, W], f16, tag="sl_n" + tw, bufs=(1 if tw else None))
            nc.scalar.activation(silu_n[:], ps_n, AF.Silu, bias=ftb_sb[:])

            ps_h2_t = pspool.tile([128, NT], f32, tag="ps_h2")
            ps_h2 = ps_h2_t[:, 0:W]
            mmi = 0
            for j in list(range(J0_L1, NB)) + list(range(J0_L1)):
                fsrc = (
                    f1d[:, j - J0_L1, :] if j >= J0_L1 else f1a[:, j, :]
                )
                nc.tensor.matmul(
                    ps_h2, d1_sb[:, j, :], fsrc[:, 0:W],
                    start=(mmi == 0), stop=False,
                )
                nc.tensor.matmul(
                    ps_h2, d1_sb[:, NB + j, :], fsrc[:, W : 2 * W],
                    start=False, stop=False,
                )
                mmi += 1
            nc.tensor.matmul(ps_h2, b1_sb[:, 0, :], silu_s[:], start=False, stop=False)
            nc.tensor.matmul(ps_h2, b1_sb[:, 1, :], silu_n[:], start=False, stop=True)

            m2d = mpool.tile([128, NB - J0_L2, W], f16, tag="m2d" + tw, bufs=(1 if tw else None))
            m2a = mpool.tile([128, J0_L2, W], f16, tag="m2a" + tw, bufs=(1 if tw else None))
            a2 = apool.tile([128, J0_L2, W], f16, tag="a2" + tw, bufs=(1 if tw else None))
            npg2 = NB - J0_L2
            in_h2 = ps_h2.unsqueeze(1).broadcast_to((128, npg2, W))
            nc.vector._custom_dve(
                TENTP, out=m2d[:], in0=in_h2,
                s0=c0l2_sb[:, J0_L2 : J0_L2 + 1], s1=c1l2_sb[:, J0_L2 : J0_L2 + 1],
                imm2=INV_H,
            )
            for j in range(J0_L2):
                nc.scalar.activation(
                    a2[:, j, :], ps_h2, AF.Abs,
                    bias=abl2_sb[:, j : j + 1], scale=INV_H,
                )
            nc.scalar.activation(
                m2a[:], a2[:], AF.Relu, bias=two_sb[:], scale=-1.0
            )
            f2d = fpool.tile([128, NB - J0_L2, W], f16, tag="f2d" + tw, bufs=(1 if tw else None))
            nc.vector._custom_dve(
                COMB, out=f2d[:], in0=m2d[:], s0=CBRT4, imm2=CBRT4,
            )
            f2a = fpool.tile([128, J0_L2, W], f16, tag="f2a" + tw, bufs=(1 if tw else None))
            nc.vector._custom_dve(
                COMB, out=f2a[:], in0=m2a[:], s0=CBRT4, imm2=CBRT4,
            )
            silu2 = slpool.tile([128, W], f16, tag="sl2" + tw, bufs=(1 if tw else None))
            nc.scalar.activation(silu2[:], ps_h2, AF.Silu, bias=0.0)

            ps_o_t = popool.tile([1, NT], f32, tag="ps_o")
            ps_o = ps_o_t[:, 0:W]
            mmi2 = 0
            for j in list(range(J0_L2, NB)) + list(range(J0_L2)):
                fsrc2 = f2d[:, j - J0_L2, :] if j >= J0_L2 else f2a[:, j, :]
                nc.tensor.matmul(
                    ps_o, d2_sb[:, j, :], fsrc2, start=(mmi2 == 0), stop=False
                )
                mmi2 += 1
            nc.tensor.matmul(ps_o, d2_sb[:, NB, :], silu2[:], start=False, stop=True)

            nc.scalar.activation(outbuf[:, sl], ps_o, AF.Identity, bias=0.0)

        nc.scalar.activation(outsig[:], outbuf[:], AF.Sigmoid, bias=0.0)
        nc.sync.dma_start(out_d[:], outsig[:])

    nc.compile()
    _CACHE["nc"] = nc
    return nc


def _host_prep(inputs):
    stm = np.asarray(inputs["stm"], dtype=np.float32)
    nstm = np.asarray(inputs["nstm"], dtype=np.float32)
    ft_w = np.asarray(inputs["ft_w"], dtype=np.float32)
    ft_b = np.asarray(inputs["ft_b"], dtype=np.float32)
    w1b = np.asarray(inputs["kan1_base_w"], dtype=np.float32)
    w1s = np.asarray(inputs["kan1_spline_w"], dtype=np.float32)
    w2b = np.asarray(inputs["kan2_base_w"], dtype=np.float32)
    w2s = np.asarray(inputs["kan2_spline_w"], dtype=np.float32)

    stmT = np.ascontiguousarray(stm.T.astype(np.float16))
    nstmT = np.ascontiguousarray(nstm.T.astype(np.float16))

    wft_np = np.ascontiguousarray(ft_w.T.reshape(KT_FT, 128, HID)).astype(np.float16)

    tmp = w1s.transpose(1, 2, 0)  # (256, 8, 128)
    d1_np = np.ascontiguousarray(
        tmp.reshape(2, 128, NB, 128).transpose(0, 2, 1, 3).reshape(2 * NB, 128, 128)
        / 6.0
    ).astype(np.float16)
    b1_np = np.ascontiguousarray(w1b.T.reshape(2, 128, HID)).astype(np.float16)

    d2_np = np.empty((NB + 1, 128, 1), dtype=np.float16)
    d2_np[:NB, :, 0] = (w2s[0].T / 6.0).astype(np.float16)
    d2_np[NB, :, 0] = w2b[0].astype(np.float16)

    bv = (ft_b.astype(np.float64) - G0) * INV_H          # (128,)
    js = np.arange(NB, dtype=np.float64)
    ones = np.ones((128, 1))
    c0l1_np = (js[None, :] - bv[:, None]).astype(np.float32)          # j - bv
    c1l1_np = (4.0 + js[None, :] - bv[:, None]).astype(np.float32)    # 4+j - bv
    abl1_np = (bv[:, None] - js[None, :] - 2.0).astype(np.float32)    # bv - j - 2
    c0l2_np = ((js[None, :] - BV2) * ones).astype(np.float32)
    c1l2_np = ((4.0 + js[None, :] - BV2) * ones).astype(np.float32)
    abl2_np = ((BV2 - js[None, :] - 2.0) * ones).astype(np.float32)
    two_np = np.full((128, 1), 2.0, dtype=np.float32)
    ftb_np = ft_b.reshape(128, 1).astype(np.float32)

    weights = dict(
        wft=wft_np, d1=d1_np, b1=b1_np, d2=d2_np,
        c0l1=c0l1_np, c1l1=c1l1_np, abl1=abl1_np,
        c0l2=c0l2_np, c1l2=c1l2_np, abl2=abl2_np,
        two=two_np, ftb=ftb_np,
    )
    return stmT, nstmT, weights


def kernel(**inputs):
    from concourse.bass_utils import run_bass_kernel_spmd

    nc = _build_module()
    stmT, nstmT, weights = _host_prep(inputs)

    in_maps = []
    for c in range(NCORES):
        sl = slice(c * BC, (c + 1) * BC)
        m = {
            "stm_t": np.ascontiguousarray(stmT[:, sl]),
            "nstm_t": np.ascontiguousarray(nstmT[:, sl]),
        }
        m.update(weights)
        in_maps.append(m)

    res = run_bass_kernel_spmd(nc, in_maps, core_ids=list(range(NCORES)))
    out = np.concatenate([r["out"].reshape(-1) for r in res.results])
    return out.reshape(B, 1).astype(np.float32)


if __name__ == "__main__":
    rng = np.random.default_rng(0)
    fake = {
        "stm": rng.random((B, IN_FT), dtype=np.float32),
        "nstm": rng.random((B, IN_FT), dtype=np.float32),
        "ft_w": (rng.standard_normal((HID, IN_FT)) * 0.02).astype(np.float32),
        "ft_b": np.zeros(HID, np.float32),
        "kan1_base_w": (rng.standard_normal((HID, 2 * HID)) * 0.05).astype(np.float32),
        "kan1_spline_w": (rng.standard_normal((HID, 2 * HID, NB)) * 0.05).astype(np.float32),
        "kan2_base_w": (rng.standard_normal((1, HID)) * 0.05).astype(np.float32),
        "kan2_spline_w": (rng.standard_normal((1, HID, NB)) * 0.05).astype(np.float32),
    }
    out = kernel(**fake)
    print("kernel out", out.shape, out.dtype, out[:5, 0])


# revision 3
# speedup vs baseline: 1.0321x; 1.0178x over previous
"""Trainium2 Bass kernel for nn_KanBoard768 (KAN forward).

Data-parallel across 8 NeuronCores: batch 32768 -> 4096 rows/core, weights
replicated, no collectives.

Math: cubic B-spline basis evaluated exactly via the bounded cardinal form
  6*B(t) = m^3 - 4*relu(m-1)^3,   m = relu(min(t, 4-t)),  t = u - j,
  u = INV_H*x + bv.
m in [0,2], 6B in [0,4]: features and spline matmuls are fp16-safe.

Engine split (v3):
 - PE: all matmuls fp16 (1 cyc/row).
 - DVE: TENT_PSUM_PAGED computes m for shifts j0..7 of a layer in ONE
   paged instruction (PageIdx), reading PSUM fp32 directly with the
   grid transform folded into per-partition constants; COMB turns the
   whole m tile into 6B in one flat pass.
 - ACT: shifts 0..j0-1 via Abs(from PSUM) + Relu(2-a) pairs, plus silu,
   all from the single `silu_and_others` act table set (no reloads).
   Sigmoid deferred to ONE final pass (one table switch per kernel).
"""

import numpy as np

GRID_SIZE, SPLINE_ORDER = 5, 3
H = 2.0 / GRID_SIZE                    # 0.4
G0 = -SPLINE_ORDER * H - 1.0           # -2.2
INV_H = 1.0 / H                        # 2.5
NB = GRID_SIZE + SPLINE_ORDER          # 8 basis functions per edge
B, IN_FT, HID = 32768, 768, 128
NCORES = 8
BC = B // NCORES                       # 4096 rows per core
NT = 512
NBT = BC // NT                         # 8 batch tiles per core
KT_FT = IN_FT // 128
CBRT4 = float(4.0 ** (1.0 / 3.0))
BV2 = -G0 * INV_H                      # 5.5 (layer-2 u bias)

# shifts j < J0 go to the ACT engine (Abs+Relu), j >= J0 to the DVE
# (single paged TENT instruction). Tuned for DVE/ACT balance.
J0_L1 = 5
J0_L2 = 4

_CACHE = {}


def _register_ops():
    import concourse.dve_ops as dve_ops
    from concourse.dve_spec import (
        Spec, Src0, C0, C1, C2, Zero, One, PageIdx, relu, sq, minn, lower,
    )
    from concourse.dve_uop import DveOpSpec

    _pg = PageIdx(Zero, One)

    def tent_ref(in0, in1, s0, s1, imm2):
        x = in0.astype(np.float32)
        S = x.shape[1] if x.ndim == 3 else 1
        pg = np.arange(S, dtype=np.float32).reshape(1, S, 1)
        s0 = np.asarray(s0, np.float32).reshape(-1, 1, 1)
        s1 = np.asarray(s1, np.float32).reshape(-1, 1, 1)
        r = x * imm2 - pg
        return np.maximum(np.minimum(r - s0, s1 - r), 0.0)

    defs = {
        # m_j = relu(min(r - C0, C1 - r)), r = x*C2 - pg, pg = page idx
        "KTENTP_ANT": (
            Spec(
                body=(lambda r: relu(minn(r - C0, C1 - r)))(Src0 * C2 - _pg),
                reference=tent_ref,
            ),
            True,
        ),
        # 6B = m^3 - relu(c*m - c)^3  (c = cbrt4)
        "KCOMB_ANT": (
            Spec(
                body=(lambda m3, w: m3 - sq(w) * w)(
                    (lambda m2: m2 * Src0)(sq(Src0)),
                    relu(Src0 * C2 - C0),
                ),
                reference=lambda in0, in1, s0, s1, imm2: (
                    in0.astype(np.float32) ** 3
                    - np.maximum(in0.astype(np.float32) * imm2 - s0, 0.0) ** 3
                ),
            ),
            False,
        ),
    }
    ops = {}
    for name, (spec, subdim) in defs.items():
        found = None
        for op in dve_ops.OPS:
            if op.name == name:
                found = op
        if found is None:
            row = dve_ops._CUSTOM_DVE_ROW_BASE + len(dve_ops.OPS)
            assert row < 0x20
            shas = {}
            for ver in ("v3", "v4"):
                try:
                    shas[ver] = DveOpSpec(
                        name=name, opcode=row, uops=lower(spec, ver=ver), rd1_en=False
                    ).sha(ver)
                except Exception:
                    pass
            found = dve_ops.DveOp(name, spec, subdim=subdim, uops_sha=shas)
            dve_ops.OPS.append(found)
            dve_ops._SUB_OPCODE_FOR_NAME[name] = row
            dve_ops.CUSTOM_DVE_SPECS[name] = spec
        ops[name] = found
    return ops


def _build_module():
    if "nc" in _CACHE:
        return _CACHE["nc"]
    from contextlib import ExitStack

    import concourse.bass as bass
    import concourse.mybir as mybir
    import concourse.tile as tile
    from concourse import bacc

    ops = _register_ops()
    TENTP, COMB = ops["KTENTP_ANT"], ops["KCOMB_ANT"]
    AF = mybir.ActivationFunctionType
    f32, f16 = mybir.dt.float32, mybir.dt.float16

    nc = bacc.Bacc("TRN2", target_bir_lowering=False, debug=False)

    stmT = nc.dram_tensor("stm_t", (IN_FT, BC), f16, kind="ExternalInput").ap()
    nstmT = nc.dram_tensor("nstm_t", (IN_FT, BC), f16, kind="ExternalInput").ap()
    wft = nc.dram_tensor("wft", (KT_FT, 128, 128), f16, kind="ExternalInput").ap()
    d1 = nc.dram_tensor("d1", (2 * NB, 128, 128), f16, kind="ExternalInput").ap()
    b1 = nc.dram_tensor("b1", (2, 128, 128), f16, kind="ExternalInput").ap()
    d2 = nc.dram_tensor("d2", (NB + 1, 128, 1), f16, kind="ExternalInput").ap()
    # constant tables: [128, 8] per use
    c0l1 = nc.dram_tensor("c0l1", (128, NB), f32, kind="ExternalInput").ap()
    c1l1 = nc.dram_tensor("c1l1", (128, NB), f32, kind="ExternalInput").ap()
    abl1 = nc.dram_tensor("abl1", (128, NB), f32, kind="ExternalInput").ap()
    c0l2 = nc.dram_tensor("c0l2", (128, NB), f32, kind="ExternalInput").ap()
    c1l2 = nc.dram_tensor("c1l2", (128, NB), f32, kind="ExternalInput").ap()
    abl2 = nc.dram_tensor("abl2", (128, NB), f32, kind="ExternalInput").ap()
    two = nc.dram_tensor("two", (128, 1), f32, kind="ExternalInput").ap()
    ftb = nc.dram_tensor("ftb", (128, 1), f32, kind="ExternalInput").ap()
    out_d = nc.dram_tensor("out", (1, BC), f16, kind="ExternalOutput").ap()

    with tile.TileContext(nc) as tc, ExitStack() as ctx:
        wpool = ctx.enter_context(tc.tile_pool(name="weights", bufs=1))
        inpool = ctx.enter_context(tc.tile_pool(name="inp", bufs=2))
        apool = ctx.enter_context(tc.tile_pool(name="abs", bufs=1))
        slpool = ctx.enter_context(tc.tile_pool(name="silu", bufs=2))
        mpool = ctx.enter_context(tc.tile_pool(name="mt", bufs=2))
        fpool = ctx.enter_context(tc.tile_pool(name="feat", bufs=2))
        opool = ctx.enter_context(tc.tile_pool(name="outb", bufs=1))
        pspool = ctx.enter_context(tc.tile_pool(name="ps", bufs=2, space="PSUM"))
        popool = ctx.enter_context(tc.tile_pool(name="pso", bufs=2, space="PSUM"))

        # prefetch the first input chunks before the bulky weight DMAs
        stmT_r = stmT.rearrange("(k p) n -> p k n", p=128)
        nstmT_r = nstmT.rearrange("(k p) n -> p k n", p=128)
        pre_xs = inpool.tile([128, KT_FT, NT // 2], f16, tag="xsh", bufs=1)
        nc.sync.dma_start(pre_xs[:], stmT_r[:, :, 0 : NT // 2])
        pre_xn = inpool.tile([128, KT_FT, NT // 2], f16, tag="xnh", bufs=1)
        nc.sync.dma_start(pre_xn[:], nstmT_r[:, :, 0 : NT // 2])

        wft_sb = wpool.tile([128, KT_FT, 128], f16)
        nc.sync.dma_start(wft_sb[:], wft.rearrange("k p m -> p k m"))
        d1_sb = wpool.tile([128, 2 * NB, 128], f16)
        nc.sync.dma_start(d1_sb[:], d1.rearrange("k p m -> p k m"))
        b1_sb = wpool.tile([128, 2, 128], f16)
        nc.sync.dma_start(b1_sb[:], b1.rearrange("k p m -> p k m"))
        d2_sb = wpool.tile([128, NB + 1, 1], f16)
        nc.sync.dma_start(d2_sb[:], d2.rearrange("k p m -> p k m"))
        c0l1_sb = wpool.tile([128, NB], f32)
        nc.sync.dma_start(c0l1_sb[:], c0l1[:])
        c1l1_sb = wpool.tile([128, NB], f32)
        nc.sync.dma_start(c1l1_sb[:], c1l1[:])
        abl1_sb = wpool.tile([128, NB], f32)
        nc.sync.dma_start(abl1_sb[:], abl1[:])
        c0l2_sb = wpool.tile([128, NB], f32)
        nc.sync.dma_start(c0l2_sb[:], c0l2[:])
        c1l2_sb = wpool.tile([128, NB], f32)
        nc.sync.dma_start(c1l2_sb[:], c1l2[:])
        abl2_sb = wpool.tile([128, NB], f32)
        nc.sync.dma_start(abl2_sb[:], abl2[:])
        two_sb = wpool.tile([128, 1], f32)
        nc.sync.dma_start(two_sb[:], two[:])
        ftb_sb = wpool.tile([128, 1], f32)
        nc.sync.dma_start(ftb_sb[:], ftb[:])

        outbuf = opool.tile([1, BC], f16)
        outsig = opool.tile([1, BC], f16)

        # chunk schedule: first and last full tiles split in half to
        # shorten pipeline fill and drain.
        chunks = [(0, NT // 2), (NT // 2, NT // 2)]
        chunks += [(i * NT, NT) for i in range(1, NBT - 1)]
        chunks += [((NBT - 1) * NT, NT // 2), ((NBT - 1) * NT + NT // 2, NT // 2)]
        for (c0, W) in chunks:
            tw = "h" if W != NT else ""
            sl = slice(c0, c0 + W)
            if c0 == 0:
                xs, xn = pre_xs, pre_xn
            else:
                xs = inpool.tile([128, KT_FT, W], f16, tag="xs" + tw, bufs=(1 if tw else None))
                nc.sync.dma_start(xs[:], stmT_r[:, :, sl])
                xn = inpool.tile([128, KT_FT, W], f16, tag="xn" + tw, bufs=(1 if tw else None))
                nc.sync.dma_start(xn[:], nstmT_r[:, :, sl])

            ps_s_t = pspool.tile([128, NT], f32, tag="ps_s")
            ps_n_t = pspool.tile([128, NT], f32, tag="ps_n")
            ps_s = ps_s_t[:, 0:W]
            ps_n = ps_n_t[:, 0:W]
            for k in range(KT_FT):
                nc.tensor.matmul(
                    ps_s, wft_sb[:, k, :], xs[:, k, :],
                    start=(k == 0), stop=(k == KT_FT - 1),
                )
            for k in range(KT_FT):
                nc.tensor.matmul(
                    ps_n, wft_sb[:, k, :], xn[:, k, :],
                    start=(k == 0), stop=(k == KT_FT - 1),
                )

            # layer-1 m/6B tiles, split into a DVE part (shifts J0..7, one
            # paged TENT) and an ACT part (shifts 0..J0-1, Abs+merged Relu)
            # so the DVE-side COMB never waits on the ACT engine.
            m1d = mpool.tile([128, NB - J0_L1, 2 * W], f16, tag="m1d" + tw, bufs=(1 if tw else None))
            m1a = mpool.tile([128, J0_L1, 2 * W], f16, tag="m1a" + tw, bufs=(1 if tw else None))
            a1 = apool.tile([128, J0_L1, 2 * W], f16, tag="a1" + tw, bufs=(1 if tw else None))
            for j in range(J0_L1):
                nc.scalar.activation(
                    a1[:, j, 0:W], ps_s, AF.Abs,
                    bias=abl1_sb[:, j : j + 1], scale=INV_H,
                )
                nc.scalar.activation(
                    a1[:, j, W : 2 * W], ps_n, AF.Abs,
                    bias=abl1_sb[:, j : j + 1], scale=INV_H,
                )
            nc.scalar.activation(
                m1a[:], a1[:], AF.Relu, bias=two_sb[:], scale=-1.0
            )
            # DVE path: shifts J0_L1..7 in one paged instruction per half
            npg = NB - J0_L1
            in_s = ps_s.unsqueeze(1).broadcast_to((128, npg, W))
            in_n = ps_n.unsqueeze(1).broadcast_to((128, npg, W))
            nc.vector._custom_dve(
                TENTP, out=m1d[:, :, 0:W], in0=in_s,
                s0=c0l1_sb[:, J0_L1 : J0_L1 + 1], s1=c1l1_sb[:, J0_L1 : J0_L1 + 1],
                imm2=INV_H,
            )
            nc.vector._custom_dve(
                TENTP, out=m1d[:, :, W : 2 * W], in0=in_n,
                s0=c0l1_sb[:, J0_L1 : J0_L1 + 1], s1=c1l1_sb[:, J0_L1 : J0_L1 + 1],
                imm2=INV_H,
            )
            f1d = fpool.tile([128, NB - J0_L1, 2 * W], f16, tag="f1d" + tw, bufs=(1 if tw else None))
            nc.vector._custom_dve(
                COMB, out=f1d[:], in0=m1d[:], s0=CBRT4, imm2=CBRT4,
            )
            f1a = fpool.tile([128, J0_L1, 2 * W], f16, tag="f1a" + tw, bufs=(1 if tw else None))
            nc.vector._custom_dve(
                COMB, out=f1a[:], in0=m1a[:], s0=CBRT4, imm2=CBRT4,
            )
            silu_s = slpool.tile([128, W], f16, tag="sl_s" + tw, bufs=(1 if tw else None))
            nc.scalar.activation(silu_s[:], ps_s, AF.Silu, bias=ftb_sb[:])
            silu_n = slpool.tile([128, W], f16, tag="sl_n" + tw, bufs=(1 if tw else None))
            nc.scalar.activation(silu_n[:], ps_n, AF.Silu, bias=ftb_sb[:])

            ps_h2_t = pspool.tile([128, NT], f32, tag="ps_h2")
            ps_h2 = ps_h2_t[:, 0:W]
            mmi = 0
            for j in list(range(J0_L1, NB)) + list(range(J0_L1)):
                fsrc = (
                    f1d[:, j - J0_L1, :] if j >= J0_L1 else f1a[:, j, :]
                )
                nc.tensor.matmul(
                    ps_h2, d1_sb[:, j, :], fsrc[:, 0:W],
                    start=(mmi == 0), stop=False,
                )
                nc.tensor.matmul(
                    ps_h2, d1_sb[:, NB + j, :], fsrc[:, W : 2 * W],
                    start=False, stop=False,
                )
                mmi += 1
            nc.tensor.matmul(ps_h2, b1_sb[:, 0, :], silu_s[:], start=False, stop=False)
            nc.tensor.matmul(ps_h2, b1_sb[:, 1, :], silu_n[:], start=False, stop=True)

            m2d = mpool.tile([128, NB - J0_L2, W], f16, tag="m2d" + tw, bufs=(1 if tw else None))
            m2a = mpool.tile([128, J0_L2, W], f16, tag="m2a" + tw, bufs=(1 if tw else None))
            a2 = apool.tile([128, J0_L2, W], f16, tag="a2" + tw, bufs=(1 if tw else None))
            npg2 = NB - J0_L2
            in_h2 = ps_h2.unsqueeze(1).broadcast_to((128, npg2, W))
            nc.vector._custom_dve(
                TENTP, out=m2d[:], in0=in_h2,
                s0=c0l2_sb[:, J0_L2 : J0_L2 + 1], s1=c1l2_sb[:, J0_L2 : J0_L2 + 1],
                imm2=INV_H,
            )
            for j in range(J0_L2):
                nc.scalar.activation(
                    a2[:, j, :], ps_h2, AF.Abs,
                    bias=abl2_sb[:, j : j + 1], scale=INV_H,
                )
            nc.scalar.activation(
                m2a[:], a2[:], AF.Relu, bias=two_sb[:], scale=-1.0
            )
            f2d = fpool.tile([128, NB - J0_L2, W], f16, tag="f2d" + tw, bufs=(1 if tw else None))
            nc.vector._custom_dve(
                COMB, out=f2d[:], in0=m2d[:], s0=CBRT4, imm2=CBRT4,
            )
            f2a = fpool.tile([128, J0_L2, W], f16, tag="f2a" + tw, bufs=(1 if tw else None))
            nc.vector._custom_dve(
                COMB, out=f2a[:], in0=m2a[:], s0=CBRT4, imm2=CBRT4,
            )
            silu2 = slpool.tile([128, W], f16, tag="sl2" + tw, bufs=(1 if tw else None))
            nc.scalar.activation(silu2[:], ps_h2, AF.Silu, bias=0.0)

            ps_o_t = popool.tile([1, NT], f32, tag="ps_o")
            ps_o = ps_o_t[:, 0:W]
            mmi2 = 0
            for j in list(range(J0_L2, NB)) + list(range(J0_L2)):
                fsrc2 = f2d[:, j - J0_L2, :] if j >= J0_L2 else f2a[:, j, :]
                nc.tensor.matmul(
                    ps_o, d2_sb[:, j, :], fsrc2, start=(mmi2 == 0), stop=False
                )
                mmi2 += 1
            nc.tensor.matmul(ps_o, d2_sb[:, NB, :], silu2[:], start=False, stop=True)

            nc.scalar.activation(outbuf[:, sl], ps_o, AF.Tanh, bias=0.0, scale=0.5)

        # sigmoid(x) = 0.5 + 0.5*tanh(x/2): tanh lives in the same ACT
        # table set as silu (no table reload); the final affine runs on DVE.
        nc.vector.tensor_scalar(
            outsig[:], outbuf[:], 0.5, 0.5, mybir.AluOpType.mult,
            mybir.AluOpType.add,
        )
        nc.sync.dma_start(out_d[:], outsig[:])

    nc.compile()
    _CACHE["nc"] = nc
    return nc


def _host_prep(inputs):
    stm = np.asarray(inputs["stm"], dtype=np.float32)
    nstm = np.asarray(inputs["nstm"], dtype=np.float32)
    ft_w = np.asarray(inputs["ft_w"], dtype=np.float32)
    ft_b = np.asarray(inputs["ft_b"], dtype=np.float32)
    w1b = np.asarray(inputs["kan1_base_w"], dtype=np.float32)
    w1s = np.asarray(inputs["kan1_spline_w"], dtype=np.float32)
    w2b = np.asarray(inputs["kan2_base_w"], dtype=np.float32)
    w2s = np.asarray(inputs["kan2_spline_w"], dtype=np.float32)

    stmT = np.ascontiguousarray(stm.T.astype(np.float16))
    nstmT = np.ascontiguousarray(nstm.T.astype(np.float16))

    wft_np = np.ascontiguousarray(ft_w.T.reshape(KT_FT, 128, HID)).astype(np.float16)

    tmp = w1s.transpose(1, 2, 0)  # (256, 8, 128)
    d1_np = np.ascontiguousarray(
        tmp.reshape(2, 128, NB, 128).transpose(0, 2, 1, 3).reshape(2 * NB, 128, 128)
        / 6.0
    ).astype(np.float16)
    b1_np = np.ascontiguousarray(w1b.T.reshape(2, 128, HID)).astype(np.float16)

    d2_np = np.empty((NB + 1, 128, 1), dtype=np.float16)
    d2_np[:NB, :, 0] = (w2s[0].T / 6.0).astype(np.float16)
    d2_np[NB, :, 0] = w2b[0].astype(np.float16)

    bv = (ft_b.astype(np.float64) - G0) * INV_H          # (128,)
    js = np.arange(NB, dtype=np.float64)
    ones = np.ones((128, 1))
    c0l1_np = (js[None, :] - bv[:, None]).astype(np.float32)          # j - bv
    c1l1_np = (4.0 + js[None, :] - bv[:, None]).astype(np.float32)    # 4+j - bv
    abl1_np = (bv[:, None] - js[None, :] - 2.0).astype(np.float32)    # bv - j - 2
    c0l2_np = ((js[None, :] - BV2) * ones).astype(np.float32)
    c1l2_np = ((4.0 + js[None, :] - BV2) * ones).astype(np.float32)
    abl2_np = ((BV2 - js[None, :] - 2.0) * ones).astype(np.float32)
    two_np = np.full((128, 1), 2.0, dtype=np.float32)
    ftb_np = ft_b.reshape(128, 1).astype(np.float32)

    weights = dict(
        wft=wft_np, d1=d1_np, b1=b1_np, d2=d2_np,
        c0l1=c0l1_np, c1l1=c1l1_np, abl1=abl1_np,
        c0l2=c0l2_np, c1l2=c1l2_np, abl2=abl2_np,
        two=two_np, ftb=ftb_np,
    )
    return stmT, nstmT, weights


def kernel(**inputs):
    from concourse.bass_utils import run_bass_kernel_spmd

    nc = _build_module()
    stmT, nstmT, weights = _host_prep(inputs)

    in_maps = []
    for c in range(NCORES):
        sl = slice(c * BC, (c + 1) * BC)
        m = {
            "stm_t": np.ascontiguousarray(stmT[:, sl]),
            "nstm_t": np.ascontiguousarray(nstmT[:, sl]),
        }
        m.update(weights)
        in_maps.append(m)

    res = run_bass_kernel_spmd(nc, in_maps, core_ids=list(range(NCORES)))
    out = np.concatenate([r["out"].reshape(-1) for r in res.results])
    return out.reshape(B, 1).astype(np.float32)


if __name__ == "__main__":
    rng = np.random.default_rng(0)
    fake = {
        "stm": rng.random((B, IN_FT), dtype=np.float32),
        "nstm": rng.random((B, IN_FT), dtype=np.float32),
        "ft_w": (rng.standard_normal((HID, IN_FT)) * 0.02).astype(np.float32),
        "ft_b": np.zeros(HID, np.float32),
        "kan1_base_w": (rng.standard_normal((HID, 2 * HID)) * 0.05).astype(np.float32),
        "kan1_spline_w": (rng.standard_normal((HID, 2 * HID, NB)) * 0.05).astype(np.float32),
        "kan2_base_w": (rng.standard_normal((1, HID)) * 0.05).astype(np.float32),
        "kan2_spline_w": (rng.standard_normal((1, HID, NB)) * 0.05).astype(np.float32),
    }
    out = kernel(**fake)
    print("kernel out", out.shape, out.dtype, out[:5, 0])


# revision 4
# speedup vs baseline: 1.0342x; 1.0021x over previous
"""Trainium2 Bass kernel for nn_KanBoard768 (KAN forward).

Data-parallel across 8 NeuronCores: batch 32768 -> 4096 rows/core, weights
replicated, no collectives.

Math: cubic B-spline basis evaluated exactly via the bounded cardinal form
  6*B(t) = m^3 - 4*relu(m-1)^3,   m = relu(min(t, 4-t)),  t = u - j,
  u = INV_H*x + bv.
m in [0,2], 6B in [0,4]: features and spline matmuls are fp16-safe.

Engine split (v3):
 - PE: all matmuls fp16 (1 cyc/row).
 - DVE: TENT_PSUM_PAGED computes m for shifts j0..7 of a layer in ONE
   paged instruction (PageIdx), reading PSUM fp32 directly with the
   grid transform folded into per-partition constants; COMB turns the
   whole m tile into 6B in one flat pass.
 - ACT: shifts 0..j0-1 via Abs(from PSUM) + Relu(2-a) pairs, plus silu,
   all from the single `silu_and_others` act table set (no reloads).
   Sigmoid deferred to ONE final pass (one table switch per kernel).
"""

import numpy as np

GRID_SIZE, SPLINE_ORDER = 5, 3
H = 2.0 / GRID_SIZE                    # 0.4
G0 = -SPLINE_ORDER * H - 1.0           # -2.2
INV_H = 1.0 / H                        # 2.5
NB = GRID_SIZE + SPLINE_ORDER          # 8 basis functions per edge
B, IN_FT, HID = 32768, 768, 128
NCORES = 8
BC = B // NCORES                       # 4096 rows per core
NT = 512
NBT = BC // NT                         # 8 batch tiles per core
KT_FT = IN_FT // 128
CBRT4 = float(4.0 ** (1.0 / 3.0))
BV2 = -G0 * INV_H                      # 5.5 (layer-2 u bias)

# shifts j < J0 go to the ACT engine (Abs+Relu), j >= J0 to the DVE
# (single paged TENT instruction). Tuned for DVE/ACT balance.
J0_L1 = 5
J0_L2 = 4

_CACHE = {}


def _register_ops():
    import concourse.dve_ops as dve_ops
    from concourse.dve_spec import (
        Spec, Src0, C0, C1, C2, Zero, One, PageIdx, relu, sq, minn, lower,
    )
    from concourse.dve_uop import DveOpSpec

    _pg = PageIdx(Zero, One)

    def tent_ref(in0, in1, s0, s1, imm2):
        x = in0.astype(np.float32)
        S = x.shape[1] if x.ndim == 3 else 1
        pg = np.arange(S, dtype=np.float32).reshape(1, S, 1)
        s0 = np.asarray(s0, np.float32).reshape(-1, 1, 1)
        s1 = np.asarray(s1, np.float32).reshape(-1, 1, 1)
        r = x * imm2 - pg
        return np.maximum(np.minimum(r - s0, s1 - r), 0.0)

    defs = {
        # m_j = relu(min(r - C0, C1 - r)), r = x*C2 - pg, pg = page idx
        "KTENTP_ANT": (
            Spec(
                body=(lambda r: relu(minn(r - C0, C1 - r)))(Src0 * C2 - _pg),
                reference=tent_ref,
            ),
            True,
        ),
        # 6B = m^3 - relu(c*m - c)^3  (c = cbrt4)
        "KCOMB_ANT": (
            Spec(
                body=(lambda m3, w: m3 - sq(w) * w)(
                    (lambda m2: m2 * Src0)(sq(Src0)),
                    relu(Src0 * C2 - C0),
                ),
                reference=lambda in0, in1, s0, s1, imm2: (
                    in0.astype(np.float32) ** 3
                    - np.maximum(in0.astype(np.float32) * imm2 - s0, 0.0) ** 3
                ),
            ),
            False,
        ),
    }
    ops = {}
    for name, (spec, subdim) in defs.items():
        found = None
        for op in dve_ops.OPS:
            if op.name == name:
                found = op
        if found is None:
            row = dve_ops._CUSTOM_DVE_ROW_BASE + len(dve_ops.OPS)
            assert row < 0x20
            shas = {}
            for ver in ("v3", "v4"):
                try:
                    shas[ver] = DveOpSpec(
                        name=name, opcode=row, uops=lower(spec, ver=ver), rd1_en=False
                    ).sha(ver)
                except Exception:
                    pass
            found = dve_ops.DveOp(name, spec, subdim=subdim, uops_sha=shas)
            dve_ops.OPS.append(found)
            dve_ops._SUB_OPCODE_FOR_NAME[name] = row
            dve_ops.CUSTOM_DVE_SPECS[name] = spec
        ops[name] = found
    return ops


def _build_module():
    if "nc" in _CACHE:
        return _CACHE["nc"]
    from contextlib import ExitStack

    import concourse.bass as bass
    import concourse.mybir as mybir
    import concourse.tile as tile
    from concourse import bacc

    ops = _register_ops()
    TENTP, COMB = ops["KTENTP_ANT"], ops["KCOMB_ANT"]
    AF = mybir.ActivationFunctionType
    f32, f16 = mybir.dt.float32, mybir.dt.float16

    nc = bacc.Bacc("TRN2", target_bir_lowering=False, debug=False)

    stmT = nc.dram_tensor("stm_t", (IN_FT, BC), f16, kind="ExternalInput").ap()
    nstmT = nc.dram_tensor("nstm_t", (IN_FT, BC), f16, kind="ExternalInput").ap()
    wft = nc.dram_tensor("wft", (KT_FT, 128, 128), f16, kind="ExternalInput").ap()
    d1 = nc.dram_tensor("d1", (2 * NB, 128, 128), f16, kind="ExternalInput").ap()
    b1 = nc.dram_tensor("b1", (2, 128, 128), f16, kind="ExternalInput").ap()
    d2 = nc.dram_tensor("d2", (NB + 1, 128, 1), f16, kind="ExternalInput").ap()
    # constant tables: [128, 8] per use
    c0l1 = nc.dram_tensor("c0l1", (128, NB), f32, kind="ExternalInput").ap()
    c1l1 = nc.dram_tensor("c1l1", (128, NB), f32, kind="ExternalInput").ap()
    abl1 = nc.dram_tensor("abl1", (128, NB), f32, kind="ExternalInput").ap()
    c0l2 = nc.dram_tensor("c0l2", (128, NB), f32, kind="ExternalInput").ap()
    c1l2 = nc.dram_tensor("c1l2", (128, NB), f32, kind="ExternalInput").ap()
    abl2 = nc.dram_tensor("abl2", (128, NB), f32, kind="ExternalInput").ap()
    two = nc.dram_tensor("two", (128, 1), f32, kind="ExternalInput").ap()
    ftb = nc.dram_tensor("ftb", (128, 1), f32, kind="ExternalInput").ap()
    out_d = nc.dram_tensor("out", (1, BC), f16, kind="ExternalOutput").ap()

    with tile.TileContext(nc) as tc, ExitStack() as ctx:
        wpool = ctx.enter_context(tc.tile_pool(name="weights", bufs=1))
        inpool = ctx.enter_context(tc.tile_pool(name="inp", bufs=2))
        apool = ctx.enter_context(tc.tile_pool(name="abs", bufs=1))
        slpool = ctx.enter_context(tc.tile_pool(name="silu", bufs=2))
        mpool = ctx.enter_context(tc.tile_pool(name="mt", bufs=2))
        fpool = ctx.enter_context(tc.tile_pool(name="feat", bufs=2))
        opool = ctx.enter_context(tc.tile_pool(name="outb", bufs=1))
        pspool = ctx.enter_context(tc.tile_pool(name="ps", bufs=2, space="PSUM"))
        popool = ctx.enter_context(tc.tile_pool(name="pso", bufs=2, space="PSUM"))

        # PE p-state warm-up: dummy matmuls with no DMA dependencies run
        # while the first input DMAs stream.
        warm_l = wpool.tile([128, 1], f16)
        nc.gpsimd.memset(warm_l[:], 0)
        warm_r = wpool.tile([128, NT // 2], f16)
        nc.gpsimd.memset(warm_r[:], 0)
        warm_ps = popool.tile([1, NT], f32, tag="ps_o")
        for _ in range(6):
            nc.tensor.matmul(warm_ps[:, 0 : NT // 2], warm_l[:], warm_r[:], start=True, stop=True)

        # prefetch the first input chunks before the bulky weight DMAs
        stmT_r = stmT.rearrange("(k p) n -> p k n", p=128)
        nstmT_r = nstmT.rearrange("(k p) n -> p k n", p=128)
        pre_xs = inpool.tile([128, KT_FT, NT // 2], f16, tag="xsh", bufs=1)
        nc.sync.dma_start(pre_xs[:], stmT_r[:, :, 0 : NT // 2])
        pre_xn = inpool.tile([128, KT_FT, NT // 2], f16, tag="xnh", bufs=1)
        nc.sync.dma_start(pre_xn[:], nstmT_r[:, :, 0 : NT // 2])

        wft_sb = wpool.tile([128, KT_FT, 128], f16)
        nc.sync.dma_start(wft_sb[:], wft.rearrange("k p m -> p k m"))
        d1_sb = wpool.tile([128, 2 * NB, 128], f16)
        nc.sync.dma_start(d1_sb[:], d1.rearrange("k p m -> p k m"))
        b1_sb = wpool.tile([128, 2, 128], f16)
        nc.sync.dma_start(b1_sb[:], b1.rearrange("k p m -> p k m"))
        d2_sb = wpool.tile([128, NB + 1, 1], f16)
        nc.sync.dma_start(d2_sb[:], d2.rearrange("k p m -> p k m"))
        c0l1_sb = wpool.tile([128, NB], f32)
        nc.sync.dma_start(c0l1_sb[:], c0l1[:])
        c1l1_sb = wpool.tile([128, NB], f32)
        nc.sync.dma_start(c1l1_sb[:], c1l1[:])
        abl1_sb = wpool.tile([128, NB], f32)
        nc.sync.dma_start(abl1_sb[:], abl1[:])
        c0l2_sb = wpool.tile([128, NB], f32)
        nc.sync.dma_start(c0l2_sb[:], c0l2[:])
        c1l2_sb = wpool.tile([128, NB], f32)
        nc.sync.dma_start(c1l2_sb[:], c1l2[:])
        abl2_sb = wpool.tile([128, NB], f32)
        nc.sync.dma_start(abl2_sb[:], abl2[:])
        two_sb = wpool.tile([128, 1], f32)
        nc.sync.dma_start(two_sb[:], two[:])
        ftb_sb = wpool.tile([128, 1], f32)
        nc.sync.dma_start(ftb_sb[:], ftb[:])

        outbuf = opool.tile([1, BC], f16)
        outsig = opool.tile([1, BC], f16)

        # chunk schedule: first and last full tiles split in half to
        # shorten pipeline fill and drain.
        chunks = [(0, NT // 2), (NT // 2, NT // 2)]
        chunks += [(i * NT, NT) for i in range(1, NBT - 1)]
        chunks += [((NBT - 1) * NT, NT // 2), ((NBT - 1) * NT + NT // 2, NT // 2)]
        pend_tanh = None
        for (c0, W) in chunks:
            tw = "h" if W != NT else ""
            sl = slice(c0, c0 + W)
            if c0 == 0:
                xs, xn = pre_xs, pre_xn
            else:
                xs = inpool.tile([128, KT_FT, W], f16, tag="xs" + tw, bufs=(1 if tw else None))
                nc.sync.dma_start(xs[:], stmT_r[:, :, sl])
                xn = inpool.tile([128, KT_FT, W], f16, tag="xn" + tw, bufs=(1 if tw else None))
                nc.sync.dma_start(xn[:], nstmT_r[:, :, sl])

            ps_s_t = pspool.tile([128, NT], f32, tag="ps_s")
            ps_n_t = pspool.tile([128, NT], f32, tag="ps_n")
            ps_s = ps_s_t[:, 0:W]
            ps_n = ps_n_t[:, 0:W]
            for k in range(KT_FT):
                nc.tensor.matmul(
                    ps_s, wft_sb[:, k, :], xs[:, k, :],
                    start=(k == 0), stop=(k == KT_FT - 1),
                )
            for k in range(KT_FT):
                nc.tensor.matmul(
                    ps_n, wft_sb[:, k, :], xn[:, k, :],
                    start=(k == 0), stop=(k == KT_FT - 1),
                )

            # layer-1 m/6B tiles, split into a DVE part (shifts J0..7, one
            # paged TENT) and an ACT part (shifts 0..J0-1, Abs+merged Relu)
            # so the DVE-side COMB never waits on the ACT engine.
            m1d = mpool.tile([128, NB - J0_L1, 2 * W], f16, tag="m1d" + tw, bufs=(1 if tw else None))
            m1a = mpool.tile([128, J0_L1, 2 * W], f16, tag="m1a" + tw, bufs=(1 if tw else None))
            a1 = apool.tile([128, J0_L1, 2 * W], f16, tag="a1" + tw, bufs=(1 if tw else None))
            for j in range(J0_L1):
                nc.scalar.activation(
                    a1[:, j, 0:W], ps_s, AF.Abs,
                    bias=abl1_sb[:, j : j + 1], scale=INV_H,
                )
                nc.scalar.activation(
                    a1[:, j, W : 2 * W], ps_n, AF.Abs,
                    bias=abl1_sb[:, j : j + 1], scale=INV_H,
                )
            nc.scalar.activation(
                m1a[:], a1[:], AF.Relu, bias=two_sb[:], scale=-1.0
            )
            if pend_tanh is not None:
                p_ps, p_sl = pend_tanh
                nc.scalar.activation(
                    outbuf[:, p_sl], p_ps, AF.Tanh, bias=0.0, scale=0.5
                )
                pend_tanh = None
            # DVE path: shifts J0_L1..7 in one paged instruction per half
            npg = NB - J0_L1
            in_s = ps_s.unsqueeze(1).broadcast_to((128, npg, W))
            in_n = ps_n.unsqueeze(1).broadcast_to((128, npg, W))
            nc.vector._custom_dve(
                TENTP, out=m1d[:, :, 0:W], in0=in_s,
                s0=c0l1_sb[:, J0_L1 : J0_L1 + 1], s1=c1l1_sb[:, J0_L1 : J0_L1 + 1],
                imm2=INV_H,
            )
            nc.vector._custom_dve(
                TENTP, out=m1d[:, :, W : 2 * W], in0=in_n,
                s0=c0l1_sb[:, J0_L1 : J0_L1 + 1], s1=c1l1_sb[:, J0_L1 : J0_L1 + 1],
                imm2=INV_H,
            )
            f1d = fpool.tile([128, NB - J0_L1, 2 * W], f16, tag="f1d" + tw, bufs=(1 if tw else None))
            nc.vector._custom_dve(
                COMB, out=f1d[:], in0=m1d[:], s0=CBRT4, imm2=CBRT4,
            )
            f1a = fpool.tile([128, J0_L1, 2 * W], f16, tag="f1a" + tw, bufs=(1 if tw else None))
            nc.vector._custom_dve(
                COMB, out=f1a[:], in0=m1a[:], s0=CBRT4, imm2=CBRT4,
            )
            silu_s = slpool.tile([128, W], f16, tag="sl_s" + tw, bufs=(1 if tw else None))
            nc.scalar.activation(silu_s[:], ps_s, AF.Silu, bias=ftb_sb[:])
            silu_n = slpool.tile([128, W], f16, tag="sl_n" + tw, bufs=(1 if tw else None))
            nc.scalar.activation(silu_n[:], ps_n, AF.Silu, bias=ftb_sb[:])

            ps_h2_t = pspool.tile([128, NT], f32, tag="ps_h2")
            ps_h2 = ps_h2_t[:, 0:W]
            mmi = 0
            for j in list(range(J0_L1, NB)) + list(range(J0_L1)):
                fsrc = (
                    f1d[:, j - J0_L1, :] if j >= J0_L1 else f1a[:, j, :]
                )
                nc.tensor.matmul(
                    ps_h2, d1_sb[:, j, :], fsrc[:, 0:W],
                    start=(mmi == 0), stop=False,
                )
                nc.tensor.matmul(
                    ps_h2, d1_sb[:, NB + j, :], fsrc[:, W : 2 * W],
                    start=False, stop=False,
                )
                mmi += 1
            nc.tensor.matmul(ps_h2, b1_sb[:, 0, :], silu_s[:], start=False, stop=False)
            nc.tensor.matmul(ps_h2, b1_sb[:, 1, :], silu_n[:], start=False, stop=True)

            m2d = mpool.tile([128, NB - J0_L2, W], f16, tag="m2d" + tw, bufs=(1 if tw else None))
            m2a = mpool.tile([128, J0_L2, W], f16, tag="m2a" + tw, bufs=(1 if tw else None))
            a2 = apool.tile([128, J0_L2, W], f16, tag="a2" + tw, bufs=(1 if tw else None))
            npg2 = NB - J0_L2
            in_h2 = ps_h2.unsqueeze(1).broadcast_to((128, npg2, W))
            nc.vector._custom_dve(
                TENTP, out=m2d[:], in0=in_h2,
                s0=c0l2_sb[:, J0_L2 : J0_L2 + 1], s1=c1l2_sb[:, J0_L2 : J0_L2 + 1],
                imm2=INV_H,
            )
            for j in range(J0_L2):
                nc.scalar.activation(
                    a2[:, j, :], ps_h2, AF.Abs,
                    bias=abl2_sb[:, j : j + 1], scale=INV_H,
                )
            nc.scalar.activation(
                m2a[:], a2[:], AF.Relu, bias=two_sb[:], scale=-1.0
            )
            f2d = fpool.tile([128, NB - J0_L2, W], f16, tag="f2d" + tw, bufs=(1 if tw else None))
            nc.vector._custom_dve(
                COMB, out=f2d[:], in0=m2d[:], s0=CBRT4, imm2=CBRT4,
            )
            f2a = fpool.tile([128, J0_L2, W], f16, tag="f2a" + tw, bufs=(1 if tw else None))
            nc.vector._custom_dve(
                COMB, out=f2a[:], in0=m2a[:], s0=CBRT4, imm2=CBRT4,
            )
            silu2 = slpool.tile([128, W], f16, tag="sl2" + tw, bufs=(1 if tw else None))
            nc.scalar.activation(silu2[:], ps_h2, AF.Silu, bias=0.0)

            ps_o_t = popool.tile([1, NT], f32, tag="ps_o")
            ps_o = ps_o_t[:, 0:W]
            mmi2 = 0
            for j in list(range(J0_L2, NB)) + list(range(J0_L2)):
                fsrc2 = f2d[:, j - J0_L2, :] if j >= J0_L2 else f2a[:, j, :]
                nc.tensor.matmul(
                    ps_o, d2_sb[:, j, :], fsrc2, start=(mmi2 == 0), stop=False
                )
                mmi2 += 1
            nc.tensor.matmul(ps_o, d2_sb[:, NB, :], silu2[:], start=False, stop=True)

            pend_tanh = (ps_o, sl)

        # sigmoid(x) = 0.5 + 0.5*tanh(x/2): tanh lives in the same ACT
        # table set as silu (no table reload); the final affine runs on DVE.
        if pend_tanh is not None:
            p_ps, p_sl = pend_tanh
            nc.scalar.activation(
                outbuf[:, p_sl], p_ps, AF.Tanh, bias=0.0, scale=0.5
            )
        CUT = BC - NT // 2
        nc.vector.tensor_scalar(
            outsig[:, 0:CUT], outbuf[:, 0:CUT], 0.5, 0.5, mybir.AluOpType.mult,
            mybir.AluOpType.add,
        )
        nc.sync.dma_start(out_d[:, 0:CUT], outsig[:, 0:CUT])
        nc.vector.tensor_scalar(
            outsig[:, CUT:BC], outbuf[:, CUT:BC], 0.5, 0.5, mybir.AluOpType.mult,
            mybir.AluOpType.add,
        )
        nc.sync.dma_start(out_d[:, CUT:BC], outsig[:, CUT:BC])

    nc.compile()
    _CACHE["nc"] = nc
    return nc


def _host_prep(inputs):
    stm = np.asarray(inputs["stm"], dtype=np.float32)
    nstm = np.asarray(inputs["nstm"], dtype=np.float32)
    ft_w = np.asarray(inputs["ft_w"], dtype=np.float32)
    ft_b = np.asarray(inputs["ft_b"], dtype=np.float32)
    w1b = np.asarray(inputs["kan1_base_w"], dtype=np.float32)
    w1s = np.asarray(inputs["kan1_spline_w"], dtype=np.float32)
    w2b = np.asarray(inputs["kan2_base_w"], dtype=np.float32)
    w2s = np.asarray(inputs["kan2_spline_w"], dtype=np.float32)

    stmT = np.ascontiguousarray(stm.T.astype(np.float16))
    nstmT = np.ascontiguousarray(nstm.T.astype(np.float16))

    wft_np = np.ascontiguousarray(ft_w.T.reshape(KT_FT, 128, HID)).astype(np.float16)

    tmp = w1s.transpose(1, 2, 0)  # (256, 8, 128)
    d1_np = np.ascontiguousarray(
        tmp.reshape(2, 128, NB, 128).transpose(0, 2, 1, 3).reshape(2 * NB, 128, 128)
        / 6.0
    ).astype(np.float16)
    b1_np = np.ascontiguousarray(w1b.T.reshape(2, 128, HID)).astype(np.float16)

    d2_np = np.empty((NB + 1, 128, 1), dtype=np.float16)
    d2_np[:NB, :, 0] = (w2s[0].T / 6.0).astype(np.float16)
    d2_np[NB, :, 0] = w2b[0].astype(np.float16)

    bv = (ft_b.astype(np.float64) - G0) * INV_H          # (128,)
    js = np.arange(NB, dtype=np.float64)
    ones = np.ones((128, 1))
    c0l1_np = (js[None, :] - bv[:, None]).astype(np.float32)          # j - bv
    c1l1_np = (4.0 + js[None, :] - bv[:, None]).astype(np.float32)    # 4+j - bv
    abl1_np = (bv[:, None] - js[None, :] - 2.0).astype(np.float32)    # bv - j - 2
    c0l2_np = ((js[None, :] - BV2) * ones).astype(np.float32)
    c1l2_np = ((4.0 + js[None, :] - BV2) * ones).astype(np.float32)
    abl2_np = ((BV2 - js[None, :] - 2.0) * ones).astype(np.float32)
    two_np = np.full((128, 1), 2.0, dtype=np.float32)
    ftb_np = ft_b.reshape(128, 1).astype(np.float32)

    weights = dict(
        wft=wft_np, d1=d1_np, b1=b1_np, d2=d2_np,
        c0l1=c0l1_np, c1l1=c1l1_np, abl1=abl1_np,
        c0l2=c0l2_np, c1l2=c1l2_np, abl2=abl2_np,
        two=two_np, ftb=ftb_np,
    )
    return stmT, nstmT, weights


def kernel(**inputs):
    from concourse.bass_utils import run_bass_kernel_spmd

    nc = _build_module()
    stmT, nstmT, weights = _host_prep(inputs)

    in_maps = []
    for c in range(NCORES):
        sl = slice(c * BC, (c + 1) * BC)
        m = {
            "stm_t": np.ascontiguousarray(stmT[:, sl]),
            "nstm_t": np.ascontiguousarray(nstmT[:, sl]),
        }
        m.update(weights)
        in_maps.append(m)

    res = run_bass_kernel_spmd(nc, in_maps, core_ids=list(range(NCORES)))
    out = np.concatenate([r["out"].reshape(-1) for r in res.results])
    return out.reshape(B, 1).astype(np.float32)


if __name__ == "__main__":
    rng = np.random.default_rng(0)
    fake = {
        "stm": rng.random((B, IN_FT), dtype=np.float32),
        "nstm": rng.random((B, IN_FT), dtype=np.float32),
        "ft_w": (rng.standard_normal((HID, IN_FT)) * 0.02).astype(np.float32),
        "ft_b": np.zeros(HID, np.float32),
        "kan1_base_w": (rng.standard_normal((HID, 2 * HID)) * 0.05).astype(np.float32),
        "kan1_spline_w": (rng.standard_normal((HID, 2 * HID, NB)) * 0.05).astype(np.float32),
        "kan2_base_w": (rng.standard_normal((1, HID)) * 0.05).astype(np.float32),
        "kan2_spline_w": (rng.standard_normal((1, HID, NB)) * 0.05).astype(np.float32),
    }
    out = kernel(**fake)
    print("kernel out", out.shape, out.dtype, out[:5, 0])
